# revision 17
# baseline (speedup 1.0000x reference)
"""DirectAU loss kernel for Trainium2, SPMD over 8 NeuronCores.

Math (see reference):
  user_e = user_table[user_id]; pos_e = item_table[pos_id]   (B=8192, D=64)
  align  = mean_i ||un_i - pn_i||^2 = 2 - (2/B) sum_i <un_i, pn_i>
  unif(x)= log( (sum_{i<j} exp(-4 + 4 <xn_i, xn_j>)) / npairs )
  out    = align + 0.5*(unif(user_e) + unif(pos_e))

Strategy (v4):
  - Cores 0-3 handle the user-table uniformity, 4-7 the pos-table one; both
    tables are concatenated so the SPMD program is identical and the table
    choice lives in the int32 gather indices.
  - Triangular block schedule per table over 8 batch chunks of 1024: core
    assignment pair {a1, a1+1}; part A covers diag(a1) + blocks to a1+4,
    part B the same shifted by one. Gathers 48 main bands + 8 align bands.
  - PE row-group pairing: xnT is stored [128, 6144] bf16 with the SAME
    64-dim data duplicated on partitions 0-63 (part A operands) and 64-127
    (part B). K=64 matmuls for A and B then occupy disjoint row groups
    (tile_position (0,0)/(64,0) auto-derived) and run CONCURRENTLY in the
    array, halving the PE streaming time vs v3. The duplication is free:
    the PE transpose reads each gathered band through a stride-0 broadcast
    AP [128, 2, 64], so one transpose writes both partition halves.
  - ACT stream: 36 uniform-bias [128, 2048] activates (PSUM ping-pong,
    pool bufs=2), each exp(4s-4[+ln.5]) with accum_out row-sums. Stages:
    T1 A-diag solo (only needs chunk0, cuts the pipeline head), T2 B-diag
    solo, then A/B-paired off-diag stages ordered by gather arrival.
  - Host sums the 8x[128,37] partials and applies the closed-form
    log/align finalization.
"""

import math

import numpy as np

import concourse.bacc as bacc
import concourse.bass as bass
import concourse.mybir as mybir
import concourse.tile as tile
from concourse import bass_utils
from concourse.masks import make_identity
from concourse.tile_rust import add_dep_helper

B = 8192
DIM = 64
NROWS = 100000
NCORES = 8
CHUNK = 1024
NCHUNK = 6  # gathered chunks per core (C0..C5)
MAIN_BANDS = NCHUNK * 8  # 48
AL_BANDS = 8
NBAND = MAIN_BANDS + AL_BANDS  # 56 gather bands
LN_HALF = math.log(0.5)
F32 = mybir.dt.float32
BF16 = mybir.dt.bfloat16
I32 = mybir.dt.int32

ALIGN_COL = 34
ACC_W = 35
PAIR = True
# Solo stages must run on the low half: long h1-only matmul bursts mixed
# with transpose-mode switches hang the PE (hardware-bisected).
SOLO_HI = False


def _emit_rsqrt(nc, pool, x_ap, out_ap, n, tag):
    """out = 1/sqrt(x) on the vector engine (bit-hack seed + 3 Newton steps)."""
    MAGIC = 0x5F3759DF
    op = mybir.AluOpType
    ti = pool.tile([128, n], I32, tag=f"{tag}_ti", name=f"{tag}_ti")
    nc.vector.tensor_scalar(
        out=ti[:], in0=x_ap.bitcast(I32), scalar1=1, scalar2=None,
        op0=op.logical_shift_right,
    )
    yi = pool.tile([128, n], I32, tag=f"{tag}_yi", name=f"{tag}_yi")
    # MAGIC - t == (t ^ -1) + (MAGIC + 1); split: ISA can't mix bitwise+arith
    nc.vector.tensor_scalar(
        out=yi[:], in0=ti[:], scalar1=-1, scalar2=None, op0=op.bitwise_xor
    )
    nc.vector.tensor_scalar(
        out=yi[:], in0=yi[:], scalar1=MAGIC + 1, scalar2=None, op0=op.add
    )
    xh = pool.tile([128, n], F32, tag=f"{tag}_xh", name=f"{tag}_xh")
    nc.vector.tensor_scalar(
        out=xh[:], in0=x_ap, scalar1=-0.5, scalar2=None, op0=op.mult
    )
    cur = yi[:].bitcast(F32)
    for it in range(2):
        t2 = pool.tile([128, n], F32, tag=f"{tag}_t2", name=f"{tag}_t2")
        nc.vector.tensor_mul(out=t2[:], in0=cur, in1=cur)
        nc.vector.tensor_mul(out=t2[:], in0=t2[:], in1=xh[:])
        nc.vector.tensor_scalar(
            out=t2[:], in0=t2[:], scalar1=1.5, scalar2=None, op0=op.add
        )
        if it == 1:
            dst_ap = out_ap
        else:
            yt = pool.tile([128, n], F32, tag=f"{tag}_y", name=f"{tag}_y{it}")
            dst_ap = yt[:]
        nc.vector.tensor_mul(out=dst_ap, in0=cur, in1=t2[:])
        cur = dst_ap
    return cur


def _body(tc, tabs, gidx, acc):
    nc = tc.nc
    op = mybir.AluOpType
    with (
        tc.tile_pool(name="persist", bufs=1) as P,
        tc.tile_pool(name="work", bufs=2) as W,
        tc.tile_pool(name="ps", bufs=2, space="PSUM") as PS,
    ):
        ident = P.tile([128, 128], F32, tag="ident")
        idx_sb = P.tile([128, NBAND], I32, tag="idx")
        nc.sync.dma_start(out=idx_sb[:], in_=gidx)

        accw = P.tile([128, ACC_W], F32, tag="accw")
        bias_o = P.tile([128, 1], F32, tag="bias_o")
        bias_d = P.tile([128, 1], F32, tag="bias_d")

        def setup_consts():
            # emitted after the first gather burst so gathers start first
            nc.gpsimd.memset(bias_o[:], -4.0)
            nc.gpsimd.memset(bias_d[:], -4.0 + LN_HALF)
            make_identity(nc, ident[:])
            # preload the exp activation-table set while gathers stream
            warm = P.tile([128, 1], F32, tag="warm")
            act_order(nc.scalar.activation(
                out=warm[:], in_=bias_o[:],
                func=mybir.ActivationFunctionType.Exp,
            ))

        # gathered rows, [128, band, DIM] band-major slots (row c*128+p)
        gath = P.tile([128, NBAND * DIM], F32, tag="gath")
        # normalized main-band rows with dims duplicated side by side
        # (band c at cols c*128; cols c*128+d and c*128+64+d both = xn[r,d]),
        # so one [128,128] PE transpose fills both partition halves of xnT
        gdup = P.tile([128, MAIN_BANDS * 2 * DIM], F32, tag="gdup")
        # xnT: dims on partitions, duplicated on both halves; cols = chunk
        # row index (chunk c at cols c*1024..c*1024+1023)
        xnT = P.tile([128, NCHUNK * CHUNK], BF16, tag="xnT")
        nsq = P.tile([128, NBAND], F32, tag="nsq")
        rinv = P.tile([128, NBAND], F32, tag="rinv")

        def gather_band(c):
            nc.gpsimd.indirect_dma_start(
                out=gath[:, c * DIM : (c + 1) * DIM],
                out_offset=None,
                in_=tabs,
                in_offset=bass.IndirectOffsetOnAxis(
                    ap=idx_sb[:, c : c + 1], axis=0
                ),
            )

        # Queue-order pinning: chain DVE normalize stages onto the previous
        # transpose's cast, and pin PE and ACT queues to emission order, so
        # the scheduler's cost model can't reorder the in-order engine
        # queues into stall-prone sequences.
        last_cast = [None]
        last_pe = [None]
        last_act = [None]

        def pe_order(inst):
            if last_pe[0] is not None:
                add_dep_helper(inst.ins, last_pe[0].ins, sync=False,
                               reason="pe order")
            last_pe[0] = inst

        def act_order(inst):
            if last_act[0] is not None:
                add_dep_helper(inst.ins, last_act[0].ins, sync=False,
                               reason="act order")
            last_act[0] = inst

        def normalize(c0, c1, tag):
            nb = c1 - c0
            sq = W.tile([128, nb * DIM], F32, tag="sq", name=f"sq_{tag}")
            g3 = gath[:, c0 * DIM : c1 * DIM].rearrange("p (c d) -> p c d", d=DIM)
            sq_inst = nc.vector.tensor_tensor(out=sq[:], in0=g3, in1=g3, op=op.mult)
            if last_cast[0] is not None:
                add_dep_helper(
                    sq_inst.ins, last_cast[0].ins, sync=False,
                    reason="dve pipeline order",
                )
            nc.vector.tensor_reduce(
                out=nsq[:, c0:c1],
                in_=sq[:].rearrange("p (c d) -> p c d", d=DIM),
                axis=mybir.AxisListType.X,
                op=op.add,
            )
            _emit_rsqrt(nc, W, nsq[:, c0:c1], rinv[:, c0:c1], nb, f"nw_{tag}")
            r3 = (
                rinv[:, c0:c1]
                .rearrange("p (c o) -> p c o", o=1)
                .to_broadcast([128, nb, DIM])
            )
            if c0 >= MAIN_BANDS:
                # align bands: single in-place normalize (f32 rows kept)
                nc.vector.tensor_tensor(out=g3, in0=g3, in1=r3, op=op.mult)
                return
            # main bands: write xn twice into the dup layout
            for k in range(2):
                dst = (
                    gdup[:, c0 * 2 * DIM : c1 * 2 * DIM]
                    .rearrange("p (c d2) -> p c d2", d2=2 * DIM)
                    [:, :, k * DIM : (k + 1) * DIM]
                )
                nc.vector.tensor_tensor(out=dst, in0=g3, in1=r3, op=op.mult)

        def transpose_bands(c0, c1):
            # 4-band groups; each [128,128] dup-band transpose fills both
            # partition halves of xnT at once
            for g in range(c0 // 4, c1 // 4):
                pt = PS.tile([128, 2048], F32, tag="ps", name=f"tp{g}")
                for k in range(4):
                    c = g * 4 + k
                    pe_order(nc.tensor.transpose(
                        out=pt[:, k * 128 : (k + 1) * 128],
                        in_=gdup[:, c * 2 * DIM : (c + 1) * 2 * DIM],
                        identity=ident[:],
                    ))
                last_cast[0] = nc.vector.tensor_copy(
                    out=xnT[:, g * 512 : (g + 1) * 512], in_=pt[:, 0:512]
                )

        hi_ok = [True]  # per-stage override: allow q=1 on the high half

        def lhs_ap(q, rt):
            hi = q == 1 and PAIR and hi_ok[0]
            half = slice(64, 128) if hi else slice(0, 64)
            c0 = q * CHUNK + rt * 128
            return xnT[half, c0 : c0 + 128]

        def rhs_ap(q, rt, j):
            hi = q == 1 and PAIR and hi_ok[0]
            half = slice(64, 128) if hi else slice(0, 64)
            if j < 8:
                cs = q * CHUNK + j * 512
            else:
                cs = (q + 4) * CHUNK + (0 if rt < 4 else 512)
            return xnT[half, cs : cs + 512]

        acol = [0]

        def emit_act(pt, w, bias_t):
            col = acol[0]
            acol[0] += 1
            act_order(nc.scalar.activation(
                out=pt[:, 0:w],
                in_=pt[:, 0:w],
                func=mybir.ActivationFunctionType.Exp,
                bias=bias_t[:],
                scale=4.0,
                accum_out=accw[:, col : col + 1],
            ))

        def mm(pt, off, q, rt, j):
            pe_order(nc.tensor.matmul(
                out=pt[:, off : off + 512],
                lhsT=lhs_ap(q, rt),
                rhs=rhs_ap(q, rt, j),
                start=True,
                stop=True,
            ))

        def diag_tile(q, rts, j, bias_t, name):
            # one 2048 tile: 4 row-tiles x one 512-col of the diag chunk
            pt = PS.tile([128, 2048], F32, tag="ps", name=name)
            for k, r in enumerate(rts):
                mm(pt, k * 512, q, r, j)
            emit_act(pt, 2048, bias_t)

        def solo_two_js(q, j0, j1):
            # 4 tiles: (2r, 2r+1) x (j0, j1) for one part, bias_o
            for r in range(0, 8, 2):
                pt = PS.tile([128, 2048], F32, tag="ps", name=f"s{q}{j0}_{r}")
                for k, (rr, j) in enumerate(
                    [(r, j0), (r, j1), (r + 1, j0), (r + 1, j1)]
                ):
                    mm(pt, k * 512, q, rr, j)
                emit_act(pt, 2048, bias_o)

        def cross_pair(ja0, ja1, jb0, jb1):
            # 8 tiles: per rt, A cols (ja0, ja1) row-group-paired with
            # B cols (jb0, jb1), bias_o
            for rt in range(8):
                pt = PS.tile([128, 2048], F32, tag="ps", name=f"x{ja0}_{rt}")
                mm(pt, 0, 0, rt, ja0)
                mm(pt, 512, 1, rt, jb0)
                mm(pt, 1024, 0, rt, ja1)
                mm(pt, 1536, 1, rt, jb1)
                emit_act(pt, 2048, bias_o)

        def stage_j8():
            # 4 tiles: (rt, rt+1) x paired A/B quadrant col, bias_o
            for rt in range(0, 8, 2):
                pt = PS.tile([128, 2048], F32, tag="ps", name=f"q8_{rt}")
                for k, r in enumerate((rt, rt + 1)):
                    mm(pt, k * 1024, 0, r, 8)
                    mm(pt, k * 1024 + 512, 1, r, 8)
                emit_act(pt, 2048, bias_o)

        def gathers(ch):
            for c in range(ch * 8, (ch + 1) * 8):
                gather_band(c)

        def unit(b0, tag):
            # normalize + transpose one 4-band group
            normalize(b0, b0 + 4, tag)
            transpose_bands(b0, b0 + 4)

        # ---- emission: data-greedy order against the serial gather
        # stream (~1.13us/band issue); 4-band units keep chunk-ready
        # latency low. Triangular diag: S00/S11 half weight, S01 full
        # (mirror S10 skipped) -> 34 activate tiles total.
        gathers(0)
        setup_consts()
        for ch in range(1, NCHUNK):
            gathers(ch)
        for c in range(MAIN_BANDS, NBAND):  # align gathers last
            gather_band(c)

        unit(0, "c0a")
        diag_tile(0, range(0, 4), 0, bias_d, "dA0")  # S00_A: bands 0-3
        unit(4, "c0b")
        diag_tile(0, range(0, 4), 1, bias_o, "dA1")  # S01_A
        diag_tile(0, range(4, 8), 1, bias_d, "dA2")  # S11_A
        unit(8, "c1a")
        hi_ok[0] = SOLO_HI
        diag_tile(1, range(0, 4), 0, bias_d, "dB0")  # S00_B: bands 8-11
        unit(12, "c1b")
        diag_tile(1, range(0, 4), 1, bias_o, "dB1")
        diag_tile(1, range(4, 8), 1, bias_d, "dB2")
        solo_two_js(0, 2, 3)  # A(j2,j3): needs only c1
        hi_ok[0] = True
        unit(16, "c2a")
        unit(20, "c2b")
        cross_pair(4, 5, 2, 3)  # A(j4,j5) x B(j2,j3): needs c2
        unit(24, "c3a")
        unit(28, "c3b")
        cross_pair(6, 7, 4, 5)  # A(j6,j7) x B(j4,j5): needs c3
        unit(32, "c4a")
        unit(36, "c4b")
        hi_ok[0] = SOLO_HI
        solo_two_js(1, 6, 7)  # B(j6,j7): needs c4
        hi_ok[0] = True
        unit(40, "c5a")
        unit(44, "c5b")
        normalize(MAIN_BANDS, NBAND, "al")
        # j8 quadrants: A chunk4 half, B chunk5 half
        stage_j8()
        al_sc = W.tile([128, AL_BANDS * DIM], F32, tag="alsc")
        un0 = (
            gdup[:, 0 : AL_BANDS * 2 * DIM]
            .rearrange("p (c d2) -> p c d2", d2=2 * DIM)[:, :, 0:DIM]
        )
        nc.vector.tensor_mul(
            out=al_sc[:].rearrange("p (c d) -> p c d", d=DIM),
            in0=un0,
            in1=gath[:, MAIN_BANDS * DIM : NBAND * DIM]
            .rearrange("p (c d) -> p c d", d=DIM),
        )
        nc.vector.tensor_reduce(
            out=accw[:, ALIGN_COL : ALIGN_COL + 1],
            in_=al_sc[:],
            axis=mybir.AxisListType.X,
            op=op.add,
        )

        nc.sync.dma_start(out=acc, in_=accw[:])


def _build():
    nc = bacc.Bacc(
        "TRN2",
        target_bir_lowering=False,
        debug=False,
        enable_asserts=False,
        num_devices=NCORES,
    )
    tabs = nc.dram_tensor("tabs", [2 * NROWS, DIM], F32, kind="ExternalInput").ap()
    gidx = nc.dram_tensor("gidx", [128, NBAND], I32, kind="ExternalInput").ap()
    acc = nc.dram_tensor("acc", [128, ACC_W], F32, kind="ExternalOutput").ap()
    with tile.TileContext(nc) as tc:
        _body(tc, tabs, gidx, acc)
    nc.compile()
    return nc


_PROG = None


def _get_prog():
    global _PROG
    if _PROG is None:
        _PROG = _build()
    return _PROG


def _core_params(m):
    """core m -> (table t, first assignment a1)."""
    t = 0 if m < 4 else 1
    j = m % 4
    a1 = 2 * j + t  # u-cores: 0,2,4,6; p-cores: 1,3,5,7
    return t, a1


def _core_gidx(uid, pid, m):
    """[128, NBAND] int32 gather indices for core m (into the concat table)."""
    t, a1 = _core_params(m)
    main_ids = [uid, pid][t]
    other_ids = [uid, pid][1 - t]
    ch = main_ids.reshape(NCORES, CHUNK)
    och = other_ids.reshape(NCORES, CHUNK)

    def h(a):  # quadrant half order for assignment a
        return 0 if a < 4 else 1

    segs = []
    for i in range(NCHUNK):
        cids = ch[(a1 + i) % NCORES].astype(np.int64) + t * NROWS
        if i == 4 and h(a1) == 1:
            cids = np.concatenate([cids[512:], cids[:512]])
        if i == 5 and h((a1 + 1) % NCORES) == 1:
            cids = np.concatenate([cids[512:], cids[:512]])
        segs.append(cids)
    # align: other table's chunk a1, batch order
    segs.append(och[a1].astype(np.int64) + (1 - t) * NROWS)
    slots = np.concatenate(segs).astype(np.int32)
    assert slots.shape == (NBAND * 128,)
    return np.ascontiguousarray(slots.reshape(NBAND, 128).T)


def _make_in_maps(user_id, pos_id, user_table, item_table):
    tabs = np.ascontiguousarray(
        np.concatenate(
            [
                np.asarray(user_table, dtype=np.float32),
                np.asarray(item_table, dtype=np.float32),
            ],
            axis=0,
        )
    )
    uid = np.asarray(user_id).astype(np.int64)
    pid = np.asarray(pos_id).astype(np.int64)
    return [
        {"tabs": tabs, "gidx": _core_gidx(uid, pid, m)} for m in range(NCORES)
    ]


def _finalize(accs):
    """accs: list of [128, ACC_W] per core -> scalar loss."""
    a = np.stack([np.asarray(x, dtype=np.float64) for x in accs])
    s_u = a[0:4, :, 0:ALIGN_COL].sum()
    s_p = a[4:8, :, 0:ALIGN_COL].sum()
    s_al = a[:, :, ALIGN_COL].sum()
    npairs = B * (B - 1) // 2
    pair_u = s_u - B / 2.0
    pair_p = s_p - B / 2.0
    unif = 0.5 * (np.log(pair_u / npairs) + np.log(pair_p / npairs))
    align = 2.0 - (2.0 / B) * s_al
    return np.asarray(align + unif, dtype=np.float32)


def _run(in_maps, trace=False, **kw):
    nc = _get_prog()
    return bass_utils.run_bass_kernel_spmd(
        nc, in_maps, core_ids=list(range(NCORES)), trace=trace, **kw
    )


def kernel(user_id, pos_id, neg_id=None, user_table=None, item_table=None):
    in_maps = _make_in_maps(user_id, pos_id, user_table, item_table)
    res = _run(in_maps, trace=False)
    return _finalize([res.results[m]["acc"] for m in range(NCORES)])


def _install_profile_hook():
    """The image's antenv lacks axon_hooks; shim it so trace=True can reach
    the NTFF profiler in libaxon_pjrt.so (same mechanism trn_boot uses)."""
    import sys
    import types

    if "antenv.axon_hooks" in sys.modules:
        return
    import antenv
    from trn_agent_boot.trn_boot import _ntff_profile_via_ctypes

    mod = types.ModuleType("antenv.axon_hooks")
    holder = [None]
    mod.set_axon_ntff_profile_hook = lambda h: holder.__setitem__(0, h)
    mod.get_axon_ntff_profile_hook = lambda: holder[0]
    sys.modules["antenv.axon_hooks"] = mod
    antenv.axon_hooks = mod
    mod.set_axon_ntff_profile_hook(
        _ntff_profile_via_ctypes("/opt/axon/libaxon_pjrt.so")
    )
    # no bucket filesystem in this container
    bass_utils.upload_artifacts = lambda tmpdir: ""


def run_profiled(user_id, pos_id, neg_id=None, user_table=None, item_table=None, **kw):
    _install_profile_hook()
    in_maps = _make_in_maps(user_id, pos_id, user_table, item_table)
    res = _run(in_maps, trace=True, **kw)
    out = _finalize([res.results[m]["acc"] for m in range(NCORES)])
    return out, res


# revision 18
# speedup vs baseline: 1.0982x; 1.0982x over previous
"""DirectAU loss kernel for Trainium2, SPMD over 8 NeuronCores.

Math (see reference):
  user_e = user_table[user_id]; pos_e = item_table[pos_id]   (B=8192, D=64)
  align  = mean_i ||un_i - pn_i||^2 = 2 - (2/B) sum_i <un_i, pn_i>
  unif(x)= log( (sum_{i<j} exp(-4 + 4 <xn_i, xn_j>)) / npairs )
  out    = align + 0.5*(unif(user_e) + unif(pos_e))

Strategy (v4):
  - Cores 0-3 handle the user-table uniformity, 4-7 the pos-table one; both
    tables are concatenated so the SPMD program is identical and the table
    choice lives in the int32 gather indices.
  - Triangular block schedule per table over 8 batch chunks of 1024: core
    assignment pair {a1, a1+1}; part A covers diag(a1) + blocks to a1+4,
    part B the same shifted by one. Gathers 48 main bands + 8 align bands.
  - PE row-group pairing: xnT is stored [128, 6144] bf16 with the SAME
    64-dim data duplicated on partitions 0-63 (part A operands) and 64-127
    (part B). K=64 matmuls for A and B then occupy disjoint row groups
    (tile_position (0,0)/(64,0) auto-derived) and run CONCURRENTLY in the
    array, halving the PE streaming time vs v3. The duplication is free:
    the PE transpose reads each gathered band through a stride-0 broadcast
    AP [128, 2, 64], so one transpose writes both partition halves.
  - ACT stream: 36 uniform-bias [128, 2048] activates (PSUM ping-pong,
    pool bufs=2), each exp(4s-4[+ln.5]) with accum_out row-sums. Stages:
    T1 A-diag solo (only needs chunk0, cuts the pipeline head), T2 B-diag
    solo, then A/B-paired off-diag stages ordered by gather arrival.
  - Host sums the 8x[128,37] partials and applies the closed-form
    log/align finalization.
"""

import math

import numpy as np

import concourse.bacc as bacc
import concourse.bass as bass
import concourse.mybir as mybir
import concourse.tile as tile
from concourse import bass_utils
from concourse.masks import make_identity
from concourse.tile_rust import add_dep_helper

B = 8192
DIM = 64
NROWS = 100000
NCORES = 8
CHUNK = 1024
NCHUNK = 6  # gathered chunks per core (C0..C5)
MAIN_BANDS = NCHUNK * 8  # 48
AL_BANDS = 8
NBAND = MAIN_BANDS + AL_BANDS  # 56 gather bands
LN_HALF = math.log(0.5)
F32 = mybir.dt.float32
BF16 = mybir.dt.bfloat16
I32 = mybir.dt.int32

ALIGN_COL = 34
ACC_W = 35
PAIR = True
# Solo stages must run on the low half: long h1-only matmul bursts mixed
# with transpose-mode switches hang the PE (hardware-bisected).
SOLO_HI = False


def _emit_rsqrt(nc, pool, x_ap, out_ap, n, tag):
    """out = 1/sqrt(x) on the vector engine (bit-hack seed + 3 Newton steps)."""
    MAGIC = 0x5F3759DF
    op = mybir.AluOpType
    ti = pool.tile([128, n], I32, tag=f"{tag}_ti", name=f"{tag}_ti")
    nc.vector.tensor_scalar(
        out=ti[:], in0=x_ap.bitcast(I32), scalar1=1, scalar2=None,
        op0=op.logical_shift_right,
    )
    yi = pool.tile([128, n], I32, tag=f"{tag}_yi", name=f"{tag}_yi")
    # MAGIC - t == (t ^ -1) + (MAGIC + 1); split: ISA can't mix bitwise+arith
    nc.vector.tensor_scalar(
        out=yi[:], in0=ti[:], scalar1=-1, scalar2=None, op0=op.bitwise_xor
    )
    nc.vector.tensor_scalar(
        out=yi[:], in0=yi[:], scalar1=MAGIC + 1, scalar2=None, op0=op.add
    )
    xh = pool.tile([128, n], F32, tag=f"{tag}_xh", name=f"{tag}_xh")
    nc.vector.tensor_scalar(
        out=xh[:], in0=x_ap, scalar1=-0.5, scalar2=None, op0=op.mult
    )
    cur = yi[:].bitcast(F32)
    for it in range(2):
        t2 = pool.tile([128, n], F32, tag=f"{tag}_t2", name=f"{tag}_t2")
        nc.vector.tensor_mul(out=t2[:], in0=cur, in1=cur)
        nc.vector.tensor_mul(out=t2[:], in0=t2[:], in1=xh[:])
        nc.vector.tensor_scalar(
            out=t2[:], in0=t2[:], scalar1=1.5, scalar2=None, op0=op.add
        )
        if it == 1:
            dst_ap = out_ap
        else:
            yt = pool.tile([128, n], F32, tag=f"{tag}_y", name=f"{tag}_y{it}")
            dst_ap = yt[:]
        nc.vector.tensor_mul(out=dst_ap, in0=cur, in1=t2[:])
        cur = dst_ap
    return cur


def _body(tc, tabs, gidx, acc):
    nc = tc.nc
    op = mybir.AluOpType
    with (
        tc.tile_pool(name="persist", bufs=1) as P,
        tc.tile_pool(name="work", bufs=2) as W,
        tc.tile_pool(name="ps", bufs=2, space="PSUM") as PS,
    ):
        ident = P.tile([128, 128], F32, tag="ident")
        idx_sb = P.tile([128, NBAND], I32, tag="idx")
        nc.sync.dma_start(out=idx_sb[:], in_=gidx)

        accw = P.tile([128, ACC_W], F32, tag="accw")
        bias_o = P.tile([128, 1], F32, tag="bias_o")
        bias_d = P.tile([128, 1], F32, tag="bias_d")

        def setup_consts():
            # emitted after the first gather burst so gathers start first
            nc.gpsimd.memset(bias_o[:], -4.0)
            nc.gpsimd.memset(bias_d[:], -4.0 + LN_HALF)
            make_identity(nc, ident[:])
            # preload the exp activation-table set while gathers stream
            warm = P.tile([128, 1], F32, tag="warm")
            act_order(nc.scalar.activation(
                out=warm[:], in_=bias_o[:],
                func=mybir.ActivationFunctionType.Exp,
            ))

        # gathered rows, [128, band, DIM] band-major slots (row c*128+p)
        gath = P.tile([128, NBAND * DIM], F32, tag="gath")
        # normalized main-band rows with dims duplicated side by side
        # (band c at cols c*128; cols c*128+d and c*128+64+d both = xn[r,d]),
        # so one [128,128] PE transpose fills both partition halves of xnT
        gdup = P.tile([128, MAIN_BANDS * 2 * DIM], F32, tag="gdup")
        # xnT: dims on partitions, duplicated on both halves; cols = chunk
        # row index (chunk c at cols c*1024..c*1024+1023)
        xnT = P.tile([128, NCHUNK * CHUNK], BF16, tag="xnT")
        nsq = P.tile([128, NBAND], F32, tag="nsq")
        rinv = P.tile([128, NBAND], F32, tag="rinv")

        def gather_band(c):
            nc.gpsimd.indirect_dma_start(
                out=gath[:, c * DIM : (c + 1) * DIM],
                out_offset=None,
                in_=tabs,
                in_offset=bass.IndirectOffsetOnAxis(
                    ap=idx_sb[:, c : c + 1], axis=0
                ),
            )

        # Queue-order pinning: chain DVE normalize stages onto the previous
        # transpose's cast, and pin PE and ACT queues to emission order, so
        # the scheduler's cost model can't reorder the in-order engine
        # queues into stall-prone sequences.
        last_cast = [None]
        last_pe = [None]
        last_act = [None]

        def pe_order(inst):
            if last_pe[0] is not None:
                add_dep_helper(inst.ins, last_pe[0].ins, sync=False,
                               reason="pe order")
            last_pe[0] = inst

        def act_order(inst):
            if last_act[0] is not None:
                add_dep_helper(inst.ins, last_act[0].ins, sync=False,
                               reason="act order")
            last_act[0] = inst

        def normalize(c0, c1, tag):
            nb = c1 - c0
            sq = W.tile([128, nb * DIM], F32, tag="sq", name=f"sq_{tag}")
            g3 = gath[:, c0 * DIM : c1 * DIM].rearrange("p (c d) -> p c d", d=DIM)
            sq_inst = nc.vector.tensor_tensor(out=sq[:], in0=g3, in1=g3, op=op.mult)
            if last_cast[0] is not None:
                add_dep_helper(
                    sq_inst.ins, last_cast[0].ins, sync=False,
                    reason="dve pipeline order",
                )
            nc.vector.tensor_reduce(
                out=nsq[:, c0:c1],
                in_=sq[:].rearrange("p (c d) -> p c d", d=DIM),
                axis=mybir.AxisListType.X,
                op=op.add,
            )
            _emit_rsqrt(nc, W, nsq[:, c0:c1], rinv[:, c0:c1], nb, f"nw_{tag}")
            r3 = (
                rinv[:, c0:c1]
                .rearrange("p (c o) -> p c o", o=1)
                .to_broadcast([128, nb, DIM])
            )
            if c0 >= MAIN_BANDS:
                # align bands: single in-place normalize (f32 rows kept)
                nc.vector.tensor_tensor(out=g3, in0=g3, in1=r3, op=op.mult)
                return
            # main bands: write xn twice into the dup layout
            for k in range(2):
                dst = (
                    gdup[:, c0 * 2 * DIM : c1 * 2 * DIM]
                    .rearrange("p (c d2) -> p c d2", d2=2 * DIM)
                    [:, :, k * DIM : (k + 1) * DIM]
                )
                nc.vector.tensor_tensor(out=dst, in0=g3, in1=r3, op=op.mult)

        def transpose_bands(c0, c1):
            # 4-band groups; each [128,128] dup-band transpose fills both
            # partition halves of xnT at once
            for g in range(c0 // 4, c1 // 4):
                pt = PS.tile([128, 2048], F32, tag="ps", name=f"tp{g}")
                for k in range(4):
                    c = g * 4 + k
                    pe_order(nc.tensor.transpose(
                        out=pt[:, k * 128 : (k + 1) * 128],
                        in_=gdup[:, c * 2 * DIM : (c + 1) * 2 * DIM],
                        identity=ident[:],
                    ))
                last_cast[0] = nc.vector.tensor_copy(
                    out=xnT[:, g * 512 : (g + 1) * 512], in_=pt[:, 0:512]
                )

        hi_ok = [True]  # per-stage override: allow q=1 on the high half

        def lhs_ap(q, rt):
            hi = q == 1 and PAIR and hi_ok[0]
            half = slice(64, 128) if hi else slice(0, 64)
            c0 = q * CHUNK + rt * 128
            return xnT[half, c0 : c0 + 128]

        def rhs_ap(q, rt, j):
            hi = q == 1 and PAIR and hi_ok[0]
            half = slice(64, 128) if hi else slice(0, 64)
            if j < 8:
                cs = q * CHUNK + j * 512
            else:
                cs = (q + 4) * CHUNK + (0 if rt < 4 else 512)
            return xnT[half, cs : cs + 512]

        acol = [0]

        def emit_act(pt, w, bias_t):
            col = acol[0]
            acol[0] += 1
            act_order(nc.scalar.activation(
                out=pt[:, 0:w],
                in_=pt[:, 0:w],
                func=mybir.ActivationFunctionType.Exp,
                bias=bias_t[:],
                scale=4.0,
                accum_out=accw[:, col : col + 1],
            ))

        def mm(pt, off, q, rt, j):
            pe_order(nc.tensor.matmul(
                out=pt[:, off : off + 512],
                lhsT=lhs_ap(q, rt),
                rhs=rhs_ap(q, rt, j),
                start=True,
                stop=True,
            ))

        def diag_tile(q, rts, j, bias_t, name):
            # one 2048 tile: 4 row-tiles x one 512-col of the diag chunk
            pt = PS.tile([128, 2048], F32, tag="ps", name=name)
            for k, r in enumerate(rts):
                mm(pt, k * 512, q, r, j)
            emit_act(pt, 2048, bias_t)

        def solo_two_js(q, j0, j1, hooks=None):
            # 4 tiles: (2r, 2r+1) x (j0, j1) for one part, bias_o
            hooks = hooks or {}
            for i, r in enumerate(range(0, 8, 2)):
                if i in hooks:
                    hooks[i]()
                pt = PS.tile([128, 2048], F32, tag="ps", name=f"s{q}{j0}_{r}")
                for k, (rr, j) in enumerate(
                    [(r, j0), (r, j1), (r + 1, j0), (r + 1, j1)]
                ):
                    mm(pt, k * 512, q, rr, j)
                emit_act(pt, 2048, bias_o)

        def cross_pair(ja0, ja1, jb0, jb1, hooks=None):
            # 8 tiles: per rt, A cols (ja0, ja1) row-group-paired with
            # B cols (jb0, jb1), bias_o
            hooks = hooks or {}
            for rt in range(8):
                if rt in hooks:
                    hooks[rt]()
                pt = PS.tile([128, 2048], F32, tag="ps", name=f"x{ja0}_{rt}")
                mm(pt, 0, 0, rt, ja0)
                mm(pt, 512, 1, rt, jb0)
                mm(pt, 1024, 0, rt, ja1)
                mm(pt, 1536, 1, rt, jb1)
                emit_act(pt, 2048, bias_o)

        def stage_j8():
            # 4 tiles: (rt, rt+1) x paired A/B quadrant col, bias_o
            for rt in range(0, 8, 2):
                pt = PS.tile([128, 2048], F32, tag="ps", name=f"q8_{rt}")
                for k, r in enumerate((rt, rt + 1)):
                    mm(pt, k * 1024, 0, r, 8)
                    mm(pt, k * 1024 + 512, 1, r, 8)
                emit_act(pt, 2048, bias_o)

        def gathers(ch):
            for c in range(ch * 8, (ch + 1) * 8):
                gather_band(c)

        def unit(b0, tag):
            # normalize + transpose one 4-band group
            normalize(b0, b0 + 4, tag)
            transpose_bands(b0, b0 + 4)

        # ---- emission: data-greedy tile order against the serial gather
        # stream (~1.13us/band issue). 4-band units (normalize+transpose)
        # are interleaved MID-stage so the PE pre-transposes the next
        # chunk while ACT still has activate backlog; units sit >=2 tiles
        # before their consumer. Triangular diag: S00/S11 half weight,
        # S01 full (mirror S10 skipped) -> 34 activate tiles.
        gathers(0)
        setup_consts()
        for ch in range(1, NCHUNK):
            gathers(ch)
        for c in range(MAIN_BANDS, NBAND):  # align gathers last
            gather_band(c)

        unit(0, "c0a")
        diag_tile(0, range(0, 4), 0, bias_d, "dA0")  # S00_A: bands 0-3
        unit(4, "c0b")
        diag_tile(0, range(0, 4), 1, bias_o, "dA1")  # S01_A
        diag_tile(0, range(4, 8), 1, bias_d, "dA2")  # S11_A
        unit(8, "c1a")
        hi_ok[0] = SOLO_HI
        diag_tile(1, range(0, 4), 0, bias_d, "dB0")  # S00_B: bands 8-11
        unit(12, "c1b")
        diag_tile(1, range(0, 4), 1, bias_o, "dB1")
        diag_tile(1, range(4, 8), 1, bias_d, "dB2")
        # A(j2,j3): needs only c1; units for c2 interleaved
        solo_two_js(0, 2, 3, hooks={0: lambda: unit(16, "c2a"),
                                    2: lambda: unit(20, "c2b")})
        hi_ok[0] = True
        # A(j4,j5) x B(j2,j3): needs c2; units for c3 interleaved
        cross_pair(4, 5, 2, 3, hooks={1: lambda: unit(24, "c3a"),
                                      3: lambda: unit(28, "c3b")})
        # A(j6,j7) x B(j4,j5): needs c3; units for c4 interleaved
        cross_pair(6, 7, 4, 5, hooks={1: lambda: unit(32, "c4a"),
                                      3: lambda: unit(36, "c4b")})
        hi_ok[0] = SOLO_HI
        # B(j6,j7): needs c4; units for c5 interleaved
        solo_two_js(1, 6, 7, hooks={0: lambda: unit(40, "c5a"),
                                    1: lambda: unit(44, "c5b")})
        hi_ok[0] = True
        normalize(MAIN_BANDS, NBAND, "al")
        # j8 quadrants: A chunk4 half, B chunk5 half
        stage_j8()
        al_sc = W.tile([128, AL_BANDS * DIM], F32, tag="alsc")
        un0 = (
            gdup[:, 0 : AL_BANDS * 2 * DIM]
            .rearrange("p (c d2) -> p c d2", d2=2 * DIM)[:, :, 0:DIM]
        )
        nc.vector.tensor_mul(
            out=al_sc[:].rearrange("p (c d) -> p c d", d=DIM),
            in0=un0,
            in1=gath[:, MAIN_BANDS * DIM : NBAND * DIM]
            .rearrange("p (c d) -> p c d", d=DIM),
        )
        nc.vector.tensor_reduce(
            out=accw[:, ALIGN_COL : ALIGN_COL + 1],
            in_=al_sc[:],
            axis=mybir.AxisListType.X,
            op=op.add,
        )

        nc.sync.dma_start(out=acc, in_=accw[:])


def _build():
    nc = bacc.Bacc(
        "TRN2",
        target_bir_lowering=False,
        debug=False,
        enable_asserts=False,
        num_devices=NCORES,
    )
    tabs = nc.dram_tensor("tabs", [2 * NROWS, DIM], F32, kind="ExternalInput").ap()
    gidx = nc.dram_tensor("gidx", [128, NBAND], I32, kind="ExternalInput").ap()
    acc = nc.dram_tensor("acc", [128, ACC_W], F32, kind="ExternalOutput").ap()
    with tile.TileContext(nc) as tc:
        _body(tc, tabs, gidx, acc)
    nc.compile()
    return nc


_PROG = None


def _get_prog():
    global _PROG
    if _PROG is None:
        _PROG = _build()
    return _PROG


def _core_params(m):
    """core m -> (table t, first assignment a1)."""
    t = 0 if m < 4 else 1
    j = m % 4
    a1 = 2 * j + t  # u-cores: 0,2,4,6; p-cores: 1,3,5,7
    return t, a1


def _core_gidx(uid, pid, m):
    """[128, NBAND] int32 gather indices for core m (into the concat table)."""
    t, a1 = _core_params(m)
    main_ids = [uid, pid][t]
    other_ids = [uid, pid][1 - t]
    ch = main_ids.reshape(NCORES, CHUNK)
    och = other_ids.reshape(NCORES, CHUNK)

    def h(a):  # quadrant half order for assignment a
        return 0 if a < 4 else 1

    segs = []
    for i in range(NCHUNK):
        cids = ch[(a1 + i) % NCORES].astype(np.int64) + t * NROWS
        if i == 4 and h(a1) == 1:
            cids = np.concatenate([cids[512:], cids[:512]])
        if i == 5 and h((a1 + 1) % NCORES) == 1:
            cids = np.concatenate([cids[512:], cids[:512]])
        segs.append(cids)
    # align: other table's chunk a1, batch order
    segs.append(och[a1].astype(np.int64) + (1 - t) * NROWS)
    slots = np.concatenate(segs).astype(np.int32)
    assert slots.shape == (NBAND * 128,)
    return np.ascontiguousarray(slots.reshape(NBAND, 128).T)


def _make_in_maps(user_id, pos_id, user_table, item_table):
    tabs = np.ascontiguousarray(
        np.concatenate(
            [
                np.asarray(user_table, dtype=np.float32),
                np.asarray(item_table, dtype=np.float32),
            ],
            axis=0,
        )
    )
    uid = np.asarray(user_id).astype(np.int64)
    pid = np.asarray(pos_id).astype(np.int64)
    return [
        {"tabs": tabs, "gidx": _core_gidx(uid, pid, m)} for m in range(NCORES)
    ]


def _finalize(accs):
    """accs: list of [128, ACC_W] per core -> scalar loss."""
    a = np.stack([np.asarray(x, dtype=np.float64) for x in accs])
    s_u = a[0:4, :, 0:ALIGN_COL].sum()
    s_p = a[4:8, :, 0:ALIGN_COL].sum()
    s_al = a[:, :, ALIGN_COL].sum()
    npairs = B * (B - 1) // 2
    pair_u = s_u - B / 2.0
    pair_p = s_p - B / 2.0
    unif = 0.5 * (np.log(pair_u / npairs) + np.log(pair_p / npairs))
    align = 2.0 - (2.0 / B) * s_al
    return np.asarray(align + unif, dtype=np.float32)


def _run(in_maps, trace=False, **kw):
    nc = _get_prog()
    return bass_utils.run_bass_kernel_spmd(
        nc, in_maps, core_ids=list(range(NCORES)), trace=trace, **kw
    )


def kernel(user_id, pos_id, neg_id=None, user_table=None, item_table=None):
    in_maps = _make_in_maps(user_id, pos_id, user_table, item_table)
    res = _run(in_maps, trace=False)
    return _finalize([res.results[m]["acc"] for m in range(NCORES)])


def _install_profile_hook():
    """The image's antenv lacks axon_hooks; shim it so trace=True can reach
    the NTFF profiler in libaxon_pjrt.so (same mechanism trn_boot uses)."""
    import sys
    import types

    if "antenv.axon_hooks" in sys.modules:
        return
    import antenv
    from trn_agent_boot.trn_boot import _ntff_profile_via_ctypes

    mod = types.ModuleType("antenv.axon_hooks")
    holder = [None]
    mod.set_axon_ntff_profile_hook = lambda h: holder.__setitem__(0, h)
    mod.get_axon_ntff_profile_hook = lambda: holder[0]
    sys.modules["antenv.axon_hooks"] = mod
    antenv.axon_hooks = mod
    mod.set_axon_ntff_profile_hook(
        _ntff_profile_via_ctypes("/opt/axon/libaxon_pjrt.so")
    )
    # no bucket filesystem in this container
    bass_utils.upload_artifacts = lambda tmpdir: ""


def run_profiled(user_id, pos_id, neg_id=None, user_table=None, item_table=None, **kw):
    _install_profile_hook()
    in_maps = _make_in_maps(user_id, pos_id, user_table, item_table)
    res = _run(in_maps, trace=True, **kw)
    out = _finalize([res.results[m]["acc"] for m in range(NCORES)])
    return out, res
